# revision 9
# baseline (speedup 1.0000x reference)
"""Trainium2 Bass kernel: HLIF spiking layer forward (LIF with soft reset).

Reference semantics per neuron, scanned over T=32 steps:
    v = v * decay + x_t ;  s = (v - vth > 0) ;  v = v - s * vth

The kernel works in threshold-scaled space w = v / vth (host prescales
xs = x / vth), so the spike test is (u > 1) and the reset subtracts 1.

Architecture (one NeuronCore per batch-pair; data-parallel over B=16 on
8 cores):

  The scan is SERIAL in t, and on real TRN2 every cross-engine hop in the
  recurrence costs ~1.4 us (semaphore+dispatch latency), so the entire
  state chain lives on the Vector engine (DVE).  The two batch items are
  interleaved as independent half-ops so every dependent same-engine pair
  is separated by an independent op (hides the SBUF write->read bubble):

  DVE : u_b  = a_b + xs_t             (tensor_tensor add, [128,512] x2)
        a_b' = (u_b - (u_b>1)) * decay (custom fused op LIF_RESET_DECAY x2)
  ACT : g = Sign(u - 1) -> {-1,+1} bf16         (spikes, off-chain)
  PE  : psum[32c:32c+32] += (W*256^kk)^T g_b    (bit-pack: 8 partitions ->
        one f32 holding 8 spike bits; 3/3/2 timesteps accumulate per slice
        at the three legal PSUM write offsets 0/32/64 -> 8 timesteps/bank)
  ACT : psum -> SBUF copy; one DMA store per group of 8 timesteps
        (16x less store traffic than storing spikes directly)

  Host decodes bits: X = (P + 255*sum(256^kk))/2 per slice, unpackbits.

Measured on HW (hardware-loop repeat-delta): ~86 us/core steady state,
~95% of which is the DVE chain floor (32 steps x 4 f32 ops + issue
overhead = 81.8 us); input DMA (16 MiB/core) fully overlaps under it.
"""

import numpy as np

B, T, C, H, W = 16, 32, 64, 32, 32
VTH_M, VTH_S, DECAY_M, DECAY_S = 0.5, 0.1, 2.0, 0.1
N_CORES = 8
B_LOC = B // N_CORES          # 2 batch items per core
P = 128
CHW = C * H * W               # 65536
FD = CHW // P                 # 512
WID = B_LOC * FD              # 1024 merged columns
GT = 8                        # timesteps packed per PSUM bank
G = T // GT                   # 4 groups
LOAD_T = 4                    # timesteps per input DMA
XP_BUFS = 6
UP_BUFS = 6
AP_BUFS = 4
GP_BUFS = 6
SP_BUFS = 4
PS_BUFS = 2

_STATE: dict = {}


# --------------------------------------------------------------------------
# Custom DVE op (registered once per process)
# --------------------------------------------------------------------------

def _get_ops():
    if "ops" in _STATE:
        return _STATE["ops"]
    from concourse import dve_ops
    from concourse.dve_spec import Spec, Src0, Src1, C0, lower, _has_src1
    from concourse.dve_uop import DveOpSpec

    def register(name, spec):
        for op in dve_ops.OPS:
            if op.name == name:
                return op
        row = dve_ops._CUSTOM_DVE_ROW_BASE + len(dve_ops.OPS)
        shas = {}
        for ver in ("v3", "v4"):
            s = DveOpSpec(
                name=name, opcode=row, uops=lower(spec, ver=ver),
                rd1_en=_has_src1(spec),
            )
            shas[ver] = s.sha(ver)
        op = dve_ops.DveOp(name, spec, subdim=False, uops_sha=shas)
        dve_ops.OPS.append(op)
        dve_ops._SUB_OPCODE_FOR_NAME[name] = row
        dve_ops.CUSTOM_DVE_SPECS[name] = spec
        return op

    # a' = (u - (u > 1)) * decay  — soft reset + leak in one DVE pass
    reset_decay = register(
        "LIF_RESET_DECAY",
        Spec(
            body=(Src0 - (Src0 > C0)) * Src1,
            reference=lambda in0, in1, s0, s1, imm2: (
                (in0.astype(np.float32) - (in0 > s0)) * in1
            ).astype(np.float32),
        ),
    )
    _STATE["ops"] = (reset_decay,)
    return _STATE["ops"]


# --------------------------------------------------------------------------
# Device kernel build
# --------------------------------------------------------------------------

def _emit_body(nc, tc, pools, tensors, reps, mybir, reset_decay, loop=False):
    f32 = mybir.dt.float32
    bf16 = mybir.dt.bfloat16
    Sign = mybir.ActivationFunctionType.Sign
    pp, xp, up, ap, gp, sp, psp = pools
    xs_d, dec_d, w_d, pk_d, dec, wpk, bias_m1 = tensors

    # First xs tiles arrive in small chunks so compute starts early.
    load_plan = [(0, 1), (1, 1), (2, 2)]
    t0n = 4
    while t0n < T:
        load_plan.append((t0n, LOAD_T))
        t0n += LOAD_T
    loads = {t0: (t0, n) for (t0, n) in load_plan}

    for r in range(reps):
        w = None   # zero state at t=0: u_0 == xs_0, no memset/add needed

        first = (r == 0) and not loop
        xt = {}
        for g in range(G):
            ps = [psp.tile([P, FD], f32, name=f"ps{r}_{b}_{g}",
                           tag=f"ps{b}") for b in range(B_LOC)]
            for k in range(GT):
                t = g * GT + k
                if first and t == 0:
                    # dec rides first: the t=0 reset op needs it
                    nc.sync.dma_start(dec, dec_d[:, :])
                    first = False
                if t in loads:
                    t0_, n_ = loads[t]
                    xl = xp.tile([P, n_, WID], f32, name=f"x{r}_{t}", tag="x")
                    nc.sync.dma_start(xl, xs_d[:, t0_:t0_ + n_, :])
                    for j in range(n_):
                        xt[t0_ + j] = xl[:, j, :]
                    if t0_ == 0 and not loop and r == 0:
                        nc.sync.dma_start(wpk, w_d[:, :])

                # interleave the two batch-halves so each dependent
                # same-engine pair is separated by an independent op
                if t == 0:
                    ut = xt[0]
                else:
                    ut = up.tile([P, WID], f32, name=f"u{r}_{t}", tag="u")
                    for h in range(B_LOC):
                        nc.vector.tensor_tensor(
                            ut[:, h * FD:(h + 1) * FD], w[h],
                            xt[t][:, h * FD:(h + 1) * FD], mybir.AluOpType.add)
                if t < T - 1:
                    wnew = []
                    for h in range(B_LOC):
                        wn = ap.tile([P, FD], f32, name=f"wn{r}_{t}_{h}",
                                     tag=f"w{h}")
                        nc.vector._custom_dve(
                            reset_decay, out=wn,
                            in0=ut[:, h * FD:(h + 1) * FD],
                            in1=dec, s0=1.0)
                        wnew.append(wn)
                    w = wnew

                gt_ = gp.tile([P, WID], bf16, name=f"g{r}_{t}", tag="g")
                nc.scalar.activation(gt_, ut, Sign, bias=bias_m1)

                # slice c (offset 32c) accumulates timesteps kk=0..2
                # (c=2: kk=0..1) with weights W*256^kk; weight columns
                # 16..31 are zero so kk=0 initializes the full slice.
                c = k // 3 if k < 6 else 2
                kk = k % 3 if k < 6 else k - 6
                last = (kk == 2) or (k == GT - 1)
                for b in range(B_LOC):
                    nc.tensor.matmul(
                        ps[b][32 * c:32 * c + 32, :],
                        wpk[:, 32 * kk:32 * (kk + 1)],
                        gt_[:, b * FD:(b + 1) * FD],
                        start=(kk == 0), stop=last)

            for b in range(B_LOC):
                st = sp.tile([96, FD], f32, name=f"st{r}_{g}_{b}", tag="st")
                nc.scalar.copy(st, ps[b][0:96, :])
                nc.sync.dma_start(pk_d[g, :, b * FD:(b + 1) * FD], st)


def _build_nc(reps=1, loop_R=None):
    import concourse.bacc as bacc
    import concourse.mybir as mybir
    from concourse.tile import TileContext

    (reset_decay,) = _get_ops()
    f32 = mybir.dt.float32
    bf16 = mybir.dt.bfloat16

    nc = bacc.Bacc(trn_type="TRN2")
    # xs partition-major: [P, T, WID]; column block b holds batch item b.
    xs_d = nc.dram_tensor("xs", [P, T, WID], f32, kind="ExternalInput")
    dec_d = nc.dram_tensor("decay", [P, FD], f32, kind="ExternalInput")
    w_d = nc.dram_tensor("wpk", [P, 96], bf16, kind="ExternalInput")
    pk_d = nc.dram_tensor("pk", [G, 96, WID], f32, kind="ExternalOutput")

    with TileContext(nc) as tc:
        with tc.tile_pool(name="pp", bufs=1) as pp, \
             tc.tile_pool(name="xp", bufs=XP_BUFS) as xp, \
             tc.tile_pool(name="up", bufs=UP_BUFS) as up, \
             tc.tile_pool(name="ap", bufs=AP_BUFS) as ap, \
             tc.tile_pool(name="gp", bufs=GP_BUFS) as gp, \
             tc.tile_pool(name="sp", bufs=SP_BUFS) as sp, \
             tc.psum_pool(name="ps", bufs=PS_BUFS) as psp:

            dec = pp.tile([P, FD], f32, name="dec", tag="dec")
            wpk = pp.tile([P, 96], bf16, name="wpk", tag="wpk")
            bias_m1 = pp.tile([P, 1], f32, name="biasm1", tag="biasm1")
            nc.gpsimd.memset(bias_m1, -1.0)

            pools = (pp, xp, up, ap, gp, sp, psp)
            tensors = (xs_d, dec_d, w_d, pk_d, dec, wpk, bias_m1)
            if loop_R is not None:
                nc.sync.dma_start(dec, dec_d[:, :])
                nc.sync.dma_start(wpk, w_d[:, :])
                with tc.For_i(0, loop_R) as _i:
                    _emit_body(nc, tc, pools, tensors, 1, mybir, reset_decay,
                               loop=True)
            else:
                _emit_body(nc, tc, pools, tensors, reps, mybir, reset_decay)
    nc.finalize()
    return nc


def _get_nc():
    nc = _STATE.get("nc")
    if nc is None:
        nc = _build_nc()
        _STATE["nc"] = nc
    return nc


# --------------------------------------------------------------------------
# Host side
# --------------------------------------------------------------------------

def _pack_weights():
    w = np.zeros((P, 96), np.float32)
    for kk in range(3):
        for p in range(P):
            w[p, 32 * kk + p // 8] = float(2 ** (p % 8 + 8 * kk))
    return w


def _prep_inputs(x, vth_raw, decay_raw):
    import ml_dtypes
    x = np.asarray(x, dtype=np.float32)
    vth_raw = np.asarray(vth_raw, dtype=np.float32)
    decay_raw = np.asarray(decay_raw, dtype=np.float32)

    vth64 = np.logaddexp(0.0, vth_raw.astype(np.float64) * VTH_S + VTH_M) + 0.01
    dec64 = 1.0 / (1.0 + np.exp(-(decay_raw.astype(np.float64) * DECAY_S + DECAY_M)))
    dec = np.clip(dec64, 0.0, 0.99).astype(np.float32)
    ivth = (1.0 / vth64).astype(np.float32)

    xs = x * ivth[None, None]                       # (B,T,C,H,W) f32
    xs_rs = xs.reshape(B, T, P, FD)
    dec_wide = np.ascontiguousarray(dec.reshape(P, FD))
    wpk = _pack_weights().astype(ml_dtypes.bfloat16)

    in_maps = []
    for kcore in range(N_CORES):
        sh = xs_rs[kcore * B_LOC:(kcore + 1) * B_LOC]   # (B_LOC, T, P, FD)
        merged = np.ascontiguousarray(
            sh.transpose(2, 1, 0, 3).reshape(P, T, WID))
        in_maps.append({"xs": merged, "decay": dec_wide, "wpk": wpk})
    return in_maps


def _decode(pk):
    """pk (G, 96, WID) f32 packed -> spikes (B_LOC, T, P, FD) f32."""
    pk = pk.reshape(G, 3, 32, WID)[:, :, :16]         # (G, c, m, WID)
    s = np.empty((G, GT, 16, 8, WID), np.uint8)
    for c in range(3):
        n_kk = 3 if c < 2 else 2
        const = 255.0 * sum(256 ** kk for kk in range(n_kk))
        y = np.rint((pk[:, c] + const) * 0.5).astype(np.int64)
        for kk in range(n_kk):
            xb = ((y >> (8 * kk)) & 0xFF).astype(np.uint8)   # (G, 16, WID)
            bits = np.unpackbits(xb[..., None], axis=-1, bitorder="little")
            s[:, 3 * c + kk] = bits.transpose(0, 1, 3, 2)
    s = s.reshape(T, P, B_LOC, FD)                    # partition p = 8m+j
    return s.transpose(2, 0, 1, 3).astype(np.float32)


def _run(in_maps, trace=False):
    from concourse.bass_utils import run_bass_kernel_spmd
    nc = _get_nc()
    return run_bass_kernel_spmd(
        nc, in_maps, core_ids=list(range(N_CORES)), trace=trace,
    )


def _assemble(res):
    out = np.empty((B, T, C, H, W), np.float32)
    for kcore in range(N_CORES):
        pk = np.asarray(res.results[kcore]["pk"], np.float32)
        out[kcore * B_LOC:(kcore + 1) * B_LOC] = _decode(pk).reshape(
            B_LOC, T, C, H, W)
    return out


def kernel(x, vth_raw, decay_raw):
    in_maps = _prep_inputs(x, vth_raw, decay_raw)
    res = _run(in_maps, trace=False)
    return _assemble(res)


def kernel_traced(x, vth_raw, decay_raw):
    in_maps = _prep_inputs(x, vth_raw, decay_raw)
    res = _run(in_maps, trace=True)
    return _assemble(res), res


# --------------------------------------------------------------------------
# HW timing (hardware-loop repeat-delta; used by test.py, not the harness)
# --------------------------------------------------------------------------

def _make_runner(nc):
    import jax
    from jax.sharding import Mesh, PartitionSpec
    from jax.experimental.shard_map import shard_map
    import concourse.mybir as mybir
    from concourse import bass2jax

    bass2jax.install_neuronx_cc_hook()

    partition_name = nc.partition_id_tensor.name if nc.partition_id_tensor else None
    in_names, out_names, out_avals, zero_outs = [], [], [], []
    for alloc in nc.m.functions[0].allocations:
        if not isinstance(alloc, mybir.MemoryLocationSet):
            continue
        name = alloc.memorylocations[0].name
        if alloc.kind == "ExternalInput":
            if name != partition_name:
                in_names.append(name)
        elif alloc.kind == "ExternalOutput":
            shape = tuple(alloc.tensor_shape)
            dtype = mybir.dt.np(alloc.dtype)
            out_names.append(name)
            out_avals.append(jax.core.ShapedArray(shape, dtype))
            zero_outs.append(np.zeros(shape, dtype))
    n_params = len(in_names)
    n_outs = len(out_avals)
    all_in_names = list(in_names) + list(out_names)
    if partition_name is not None:
        all_in_names.append(partition_name)

    def _body(*args):
        operands = list(args)
        if partition_name is not None:
            operands.append(bass2jax.partition_id_tensor())
        outs = bass2jax._bass_exec_p.bind(
            *operands,
            out_avals=tuple(out_avals),
            in_names=tuple(all_in_names),
            out_names=tuple(out_names),
            lowering_input_output_aliases=(),
            sim_require_finite=True,
            sim_require_nnan=True,
            nc=nc,
        )
        return tuple(outs)

    devices = jax.devices()[:N_CORES]
    mesh = Mesh(np.asarray(devices), ("core",))
    in_specs = (PartitionSpec("core"),) * (n_params + n_outs)
    out_specs = (PartitionSpec("core"),) * n_outs
    sharded = jax.jit(
        shard_map(_body, mesh=mesh, in_specs=in_specs, out_specs=out_specs,
                  check_rep=False),
        keep_unused=True,
    )

    from jax.sharding import NamedSharding
    zero_sharding = NamedSharding(mesh, PartitionSpec("core"))
    zero_cache = []

    def run(concat_inputs_by_name):
        if not zero_cache:
            zero_cache.extend(
                jax.device_put(
                    np.zeros((N_CORES * z.shape[0], *z.shape[1:]), z.dtype),
                    zero_sharding,
                )
                for z in zero_outs
            )
        args = [concat_inputs_by_name[n] for n in in_names]
        args += zero_cache
        outs = sharded(*args)
        return outs, out_names

    run.mesh = mesh
    run.in_names = in_names
    run.out_names = out_names
    return run


def measure_hw_ns(x, vth_raw, decay_raw, r_lo=4, r_hi=1028, n_calls=8):
    """Steady-state per-iteration device time: the same kernel wrapped in a
    For_i hardware loop run at R=r_lo and R=r_hi; (minwall delta)/(R delta)
    cancels the ~+-15 ms axon dispatch noise (signal ~50 ms at R=516)."""
    import time
    import jax
    from jax.sharding import NamedSharding, PartitionSpec

    in_maps = _prep_inputs(x, vth_raw, decay_raw)
    concat = {
        n: np.concatenate([np.asarray(m[n]) for m in in_maps], axis=0)
        for n in in_maps[0]
    }
    mins = {}
    for R in (r_lo, r_hi):
        nc = _build_nc(loop_R=R)
        run = _make_runner(nc)
        sh = NamedSharding(run.mesh, PartitionSpec("core"))
        dev_in = {n: jax.device_put(concat[n], sh) for n in run.in_names}
        outs, _ = run(dev_in)           # warmup + compile
        jax.block_until_ready(outs)
        ts = []
        for _ in range(n_calls):
            t0 = time.perf_counter()
            outs, _ = run(dev_in)
            jax.block_until_ready(outs)
            ts.append(time.perf_counter() - t0)
        mins[R] = min(ts)
        print(f"  R={R}: min={min(ts)*1e3:.2f} ms  all={[f'{t*1e3:.1f}' for t in ts]}")
    ns = (mins[r_hi] - mins[r_lo]) / (r_hi - r_lo) * 1e9
    return ns, mins


# revision 12
# speedup vs baseline: 1.0006x; 1.0006x over previous
"""Trainium2 Bass kernel: HLIF spiking layer forward (LIF with soft reset).

Reference semantics per neuron, scanned over T=32 steps:
    v = v * decay + x_t ;  s = (v - vth > 0) ;  v = v - s * vth

The kernel works in threshold-scaled space w = v / vth (host prescales
xs = x / vth), so the spike test is (u > 1) and the reset subtracts 1.

Architecture (one NeuronCore per batch-pair; data-parallel over B=16 on
8 cores):

  The scan is SERIAL in t, and on real TRN2 every cross-engine hop in the
  recurrence costs ~1.4 us (semaphore+dispatch latency), so the entire
  state chain lives on the Vector engine (DVE).  The two batch items are
  interleaved as independent half-ops so every dependent same-engine pair
  is separated by an independent op (hides the SBUF write->read bubble):

  DVE : u_b  = a_b + xs_t             (tensor_tensor add, [128,512] x2)
        a_b' = (u_b - (u_b>1)) * decay (custom fused op LIF_RESET_DECAY x2)
  ACT : g = Sign(u - 1) -> {-1,+1} bf16         (spikes, off-chain)
  PE  : psum[32c:32c+32] += (W*256^kk)^T g_b    (bit-pack: 8 partitions ->
        one f32 holding 8 spike bits; 3/3/2 timesteps accumulate per slice
        at the three legal PSUM write offsets 0/32/64 -> 8 timesteps/bank)
  ACT : psum -> SBUF copy; one DMA store per group of 8 timesteps
        (16x less store traffic than storing spikes directly)

  Host decodes bits: X = (P + 255*sum(256^kk))/2 per slice, unpackbits.

Measured on HW (hardware-loop repeat-delta): ~86 us/core steady state,
~95% of which is the DVE chain floor (32 steps x 4 f32 ops + issue
overhead = 81.8 us); input DMA (16 MiB/core) fully overlaps under it.
"""

import numpy as np

B, T, C, H, W = 16, 32, 64, 32, 32
VTH_M, VTH_S, DECAY_M, DECAY_S = 0.5, 0.1, 2.0, 0.1
N_CORES = 8
B_LOC = B // N_CORES          # 2 batch items per core
P = 128
CHW = C * H * W               # 65536
FD = CHW // P                 # 512
WID = B_LOC * FD              # 1024 merged columns
GT = 8                        # timesteps packed per PSUM bank
G = T // GT                   # 4 groups
LOAD_T = 4                    # timesteps per input DMA
XP_BUFS = 6
UP_BUFS = 6
AP_BUFS = 4
GP_BUFS = 6
SP_BUFS = 4
PS_BUFS = 2

_STATE: dict = {}


# --------------------------------------------------------------------------
# Custom DVE op (registered once per process)
# --------------------------------------------------------------------------

def _get_ops():
    if "ops" in _STATE:
        return _STATE["ops"]
    from concourse import dve_ops
    from concourse.dve_spec import Spec, Src0, Src1, C0, lower, _has_src1
    from concourse.dve_uop import DveOpSpec

    def register(name, spec):
        for op in dve_ops.OPS:
            if op.name == name:
                return op
        row = dve_ops._CUSTOM_DVE_ROW_BASE + len(dve_ops.OPS)
        shas = {}
        for ver in ("v3", "v4"):
            s = DveOpSpec(
                name=name, opcode=row, uops=lower(spec, ver=ver),
                rd1_en=_has_src1(spec),
            )
            shas[ver] = s.sha(ver)
        op = dve_ops.DveOp(name, spec, subdim=False, uops_sha=shas)
        dve_ops.OPS.append(op)
        dve_ops._SUB_OPCODE_FOR_NAME[name] = row
        dve_ops.CUSTOM_DVE_SPECS[name] = spec
        return op

    # a' = (u - (u > 1)) * decay  — soft reset + leak in one DVE pass
    reset_decay = register(
        "LIF_RESET_DECAY",
        Spec(
            body=(Src0 - (Src0 > C0)) * Src1,
            reference=lambda in0, in1, s0, s1, imm2: (
                (in0.astype(np.float32) - (in0 > s0)) * in1
            ).astype(np.float32),
        ),
    )
    _STATE["ops"] = (reset_decay,)
    return _STATE["ops"]


# --------------------------------------------------------------------------
# Device kernel build
# --------------------------------------------------------------------------

def _emit_body(nc, tc, pools, tensors, reps, mybir, reset_decay, loop=False):
    f32 = mybir.dt.float32
    bf16 = mybir.dt.bfloat16
    Sign = mybir.ActivationFunctionType.Sign
    pp, xp, up, ap, gp, sp, psp = pools
    xs_d, dec_d, w_d, pk_d, dec, wpk, bias_m1 = tensors

    # First xs tiles arrive in small chunks so compute starts early.
    load_plan = [(0, 1), (1, 1), (2, 2)]
    t0n = 4
    while t0n < T:
        load_plan.append((t0n, LOAD_T))
        t0n += LOAD_T
    loads = {t0: (t0, n, i) for i, (t0, n) in enumerate(load_plan)}

    for r in range(reps):
        w = None   # zero state at t=0: u_0 == xs_0, no memset/add needed

        first = (r == 0) and not loop
        xt = {}
        for g in range(G):
            ps = [psp.tile([P, FD], f32, name=f"ps{r}_{b}_{g}",
                           tag=f"ps{b}") for b in range(B_LOC)]
            for k in range(GT):
                t = g * GT + k
                if first and t == 0:
                    # dec rides first: the t=0 reset op needs it
                    nc.sync.dma_start(dec, dec_d[:, :])
                    first = False
                if t in loads:
                    t0_, n_, li_ = loads[t]
                    xl = xp.tile([P, n_, WID], f32, name=f"x{r}_{t}", tag="x")
                    nc.sync.dma_start(xl, xs_d[:, t0_:t0_ + n_, :])
                    for j in range(n_):
                        xt[t0_ + j] = xl[:, j, :]
                    if t0_ == 0 and not loop and r == 0:
                        nc.sync.dma_start(wpk, w_d[:, :])

                # interleave the two batch-halves so each dependent
                # same-engine pair is separated by an independent op
                if t == 0:
                    ut = xt[0]
                else:
                    ut = up.tile([P, WID], f32, name=f"u{r}_{t}", tag="u")
                    for h in range(B_LOC):
                        nc.vector.tensor_tensor(
                            ut[:, h * FD:(h + 1) * FD], w[h],
                            xt[t][:, h * FD:(h + 1) * FD], mybir.AluOpType.add)
                if t < T - 1:
                    wnew = []
                    for h in range(B_LOC):
                        wn = ap.tile([P, FD], f32, name=f"wn{r}_{t}_{h}",
                                     tag=f"w{h}")
                        nc.vector._custom_dve(
                            reset_decay, out=wn,
                            in0=ut[:, h * FD:(h + 1) * FD],
                            in1=dec, s0=1.0)
                        wnew.append(wn)
                    w = wnew

                gt_ = gp.tile([P, WID], bf16, name=f"g{r}_{t}", tag="g")
                nc.scalar.activation(gt_, ut, Sign, bias=bias_m1)

                # slice c (offset 32c) accumulates timesteps kk=0..2
                # (c=2: kk=0..1) with weights W*256^kk; weight columns
                # 16..31 are zero so kk=0 initializes the full slice.
                c = k // 3 if k < 6 else 2
                kk = k % 3 if k < 6 else k - 6
                last = (kk == 2) or (k == GT - 1)
                for b in range(B_LOC):
                    nc.tensor.matmul(
                        ps[b][32 * c:32 * c + 32, :],
                        wpk[:, 32 * kk:32 * (kk + 1)],
                        gt_[:, b * FD:(b + 1) * FD],
                        start=(kk == 0), stop=last)

            for b in range(B_LOC):
                st = sp.tile([96, FD], f32, name=f"st{r}_{g}_{b}", tag="st")
                nc.scalar.copy(st, ps[b][0:96, :])
                nc.sync.dma_start(pk_d[g, :, b * FD:(b + 1) * FD], st)


def _build_nc(reps=1, loop_R=None):
    import concourse.bacc as bacc
    import concourse.mybir as mybir
    from concourse.tile import TileContext

    (reset_decay,) = _get_ops()
    f32 = mybir.dt.float32
    bf16 = mybir.dt.bfloat16

    nc = bacc.Bacc(trn_type="TRN2")
    # xs partition-major: [P, T, WID]; column block b holds batch item b.
    xs_d = nc.dram_tensor("xs", [P, T, WID], f32, kind="ExternalInput")
    dec_d = nc.dram_tensor("decay", [P, FD], f32, kind="ExternalInput")
    w_d = nc.dram_tensor("wpk", [P, 96], bf16, kind="ExternalInput")
    pk_d = nc.dram_tensor("pk", [G, 96, WID], f32, kind="ExternalOutput")

    with TileContext(nc) as tc:
        with tc.tile_pool(name="pp", bufs=1) as pp, \
             tc.tile_pool(name="xp", bufs=XP_BUFS) as xp, \
             tc.tile_pool(name="up", bufs=UP_BUFS) as up, \
             tc.tile_pool(name="ap", bufs=AP_BUFS) as ap, \
             tc.tile_pool(name="gp", bufs=GP_BUFS) as gp, \
             tc.tile_pool(name="sp", bufs=SP_BUFS) as sp, \
             tc.psum_pool(name="ps", bufs=PS_BUFS) as psp:

            dec = pp.tile([P, FD], f32, name="dec", tag="dec")
            wpk = pp.tile([P, 96], bf16, name="wpk", tag="wpk")
            bias_m1 = pp.tile([P, 1], f32, name="biasm1", tag="biasm1")
            nc.gpsimd.memset(bias_m1, -1.0)

            pools = (pp, xp, up, ap, gp, sp, psp)
            tensors = (xs_d, dec_d, w_d, pk_d, dec, wpk, bias_m1)
            if loop_R is not None:
                nc.sync.dma_start(dec, dec_d[:, :])
                nc.sync.dma_start(wpk, w_d[:, :])
                with tc.For_i(0, loop_R) as _i:
                    _emit_body(nc, tc, pools, tensors, 1, mybir, reset_decay,
                               loop=True)
            else:
                _emit_body(nc, tc, pools, tensors, reps, mybir, reset_decay)
    nc.finalize()
    return nc


def _get_nc():
    nc = _STATE.get("nc")
    if nc is None:
        nc = _build_nc()
        _STATE["nc"] = nc
    return nc


# --------------------------------------------------------------------------
# Host side
# --------------------------------------------------------------------------

def _pack_weights():
    w = np.zeros((P, 96), np.float32)
    for kk in range(3):
        for p in range(P):
            w[p, 32 * kk + p // 8] = float(2 ** (p % 8 + 8 * kk))
    return w


def _prep_inputs(x, vth_raw, decay_raw):
    import ml_dtypes
    x = np.asarray(x, dtype=np.float32)
    vth_raw = np.asarray(vth_raw, dtype=np.float32)
    decay_raw = np.asarray(decay_raw, dtype=np.float32)

    vth64 = np.logaddexp(0.0, vth_raw.astype(np.float64) * VTH_S + VTH_M) + 0.01
    dec64 = 1.0 / (1.0 + np.exp(-(decay_raw.astype(np.float64) * DECAY_S + DECAY_M)))
    dec = np.clip(dec64, 0.0, 0.99).astype(np.float32)
    ivth = (1.0 / vth64).astype(np.float32)

    xs = x * ivth[None, None]                       # (B,T,C,H,W) f32
    xs_rs = xs.reshape(B, T, P, FD)
    dec_wide = np.ascontiguousarray(dec.reshape(P, FD))
    wpk = _pack_weights().astype(ml_dtypes.bfloat16)

    in_maps = []
    for kcore in range(N_CORES):
        sh = xs_rs[kcore * B_LOC:(kcore + 1) * B_LOC]   # (B_LOC, T, P, FD)
        merged = np.ascontiguousarray(
            sh.transpose(2, 1, 0, 3).reshape(P, T, WID))
        in_maps.append({"xs": merged, "decay": dec_wide, "wpk": wpk})
    return in_maps


def _decode(pk):
    """pk (G, 96, WID) f32 packed -> spikes (B_LOC, T, P, FD) f32."""
    pk = pk.reshape(G, 3, 32, WID)[:, :, :16]         # (G, c, m, WID)
    s = np.empty((G, GT, 16, 8, WID), np.uint8)
    for c in range(3):
        n_kk = 3 if c < 2 else 2
        const = 255.0 * sum(256 ** kk for kk in range(n_kk))
        y = np.rint((pk[:, c] + const) * 0.5).astype(np.int64)
        for kk in range(n_kk):
            xb = ((y >> (8 * kk)) & 0xFF).astype(np.uint8)   # (G, 16, WID)
            bits = np.unpackbits(xb[..., None], axis=-1, bitorder="little")
            s[:, 3 * c + kk] = bits.transpose(0, 1, 3, 2)
    s = s.reshape(T, P, B_LOC, FD)                    # partition p = 8m+j
    return s.transpose(2, 0, 1, 3).astype(np.float32)


def _run(in_maps, trace=False):
    from concourse.bass_utils import run_bass_kernel_spmd
    nc = _get_nc()
    return run_bass_kernel_spmd(
        nc, in_maps, core_ids=list(range(N_CORES)), trace=trace,
    )


def _assemble(res):
    out = np.empty((B, T, C, H, W), np.float32)
    for kcore in range(N_CORES):
        pk = np.asarray(res.results[kcore]["pk"], np.float32)
        out[kcore * B_LOC:(kcore + 1) * B_LOC] = _decode(pk).reshape(
            B_LOC, T, C, H, W)
    return out


def kernel(x, vth_raw, decay_raw):
    in_maps = _prep_inputs(x, vth_raw, decay_raw)
    res = _run(in_maps, trace=False)
    return _assemble(res)


def kernel_traced(x, vth_raw, decay_raw):
    in_maps = _prep_inputs(x, vth_raw, decay_raw)
    res = _run(in_maps, trace=True)
    return _assemble(res), res


# --------------------------------------------------------------------------
# HW timing (hardware-loop repeat-delta; used by test.py, not the harness)
# --------------------------------------------------------------------------

def _make_runner(nc):
    import jax
    from jax.sharding import Mesh, PartitionSpec
    from jax.experimental.shard_map import shard_map
    import concourse.mybir as mybir
    from concourse import bass2jax

    bass2jax.install_neuronx_cc_hook()

    partition_name = nc.partition_id_tensor.name if nc.partition_id_tensor else None
    in_names, out_names, out_avals, zero_outs = [], [], [], []
    for alloc in nc.m.functions[0].allocations:
        if not isinstance(alloc, mybir.MemoryLocationSet):
            continue
        name = alloc.memorylocations[0].name
        if alloc.kind == "ExternalInput":
            if name != partition_name:
                in_names.append(name)
        elif alloc.kind == "ExternalOutput":
            shape = tuple(alloc.tensor_shape)
            dtype = mybir.dt.np(alloc.dtype)
            out_names.append(name)
            out_avals.append(jax.core.ShapedArray(shape, dtype))
            zero_outs.append(np.zeros(shape, dtype))
    n_params = len(in_names)
    n_outs = len(out_avals)
    all_in_names = list(in_names) + list(out_names)
    if partition_name is not None:
        all_in_names.append(partition_name)

    def _body(*args):
        operands = list(args)
        if partition_name is not None:
            operands.append(bass2jax.partition_id_tensor())
        outs = bass2jax._bass_exec_p.bind(
            *operands,
            out_avals=tuple(out_avals),
            in_names=tuple(all_in_names),
            out_names=tuple(out_names),
            lowering_input_output_aliases=(),
            sim_require_finite=True,
            sim_require_nnan=True,
            nc=nc,
        )
        return tuple(outs)

    devices = jax.devices()[:N_CORES]
    mesh = Mesh(np.asarray(devices), ("core",))
    in_specs = (PartitionSpec("core"),) * (n_params + n_outs)
    out_specs = (PartitionSpec("core"),) * n_outs
    sharded = jax.jit(
        shard_map(_body, mesh=mesh, in_specs=in_specs, out_specs=out_specs,
                  check_rep=False),
        keep_unused=True,
    )

    from jax.sharding import NamedSharding
    zero_sharding = NamedSharding(mesh, PartitionSpec("core"))
    zero_cache = []

    def run(concat_inputs_by_name):
        if not zero_cache:
            zero_cache.extend(
                jax.device_put(
                    np.zeros((N_CORES * z.shape[0], *z.shape[1:]), z.dtype),
                    zero_sharding,
                )
                for z in zero_outs
            )
        args = [concat_inputs_by_name[n] for n in in_names]
        args += zero_cache
        outs = sharded(*args)
        return outs, out_names

    run.mesh = mesh
    run.in_names = in_names
    run.out_names = out_names
    return run


def measure_hw_ns(x, vth_raw, decay_raw, r_lo=4, r_hi=1028, n_calls=8):
    """Steady-state per-iteration device time: the same kernel wrapped in a
    For_i hardware loop run at R=r_lo and R=r_hi; (minwall delta)/(R delta)
    cancels the ~+-15 ms axon dispatch noise (signal ~50 ms at R=516)."""
    import time
    import jax
    from jax.sharding import NamedSharding, PartitionSpec

    in_maps = _prep_inputs(x, vth_raw, decay_raw)
    concat = {
        n: np.concatenate([np.asarray(m[n]) for m in in_maps], axis=0)
        for n in in_maps[0]
    }
    mins = {}
    for R in (r_lo, r_hi):
        nc = _build_nc(loop_R=R)
        run = _make_runner(nc)
        sh = NamedSharding(run.mesh, PartitionSpec("core"))
        dev_in = {n: jax.device_put(concat[n], sh) for n in run.in_names}
        outs, _ = run(dev_in)           # warmup + compile
        jax.block_until_ready(outs)
        ts = []
        for _ in range(n_calls):
            t0 = time.perf_counter()
            outs, _ = run(dev_in)
            jax.block_until_ready(outs)
            ts.append(time.perf_counter() - t0)
        mins[R] = min(ts)
        print(f"  R={R}: min={min(ts)*1e3:.2f} ms  all={[f'{t*1e3:.1f}' for t in ts]}")
    ns = (mins[r_hi] - mins[r_lo]) / (r_hi - r_lo) * 1e9
    return ns, mins


# revision 14
# speedup vs baseline: 1.0021x; 1.0015x over previous
"""Trainium2 Bass kernel: HLIF spiking layer forward (LIF with soft reset).

Reference semantics per neuron, scanned over T=32 steps:
    v = v * decay + x_t ;  s = (v - vth > 0) ;  v = v - s * vth

The kernel works in threshold-scaled space w = v / vth (host prescales
xs = x / vth), so the spike test is (u > 1) and the reset subtracts 1.

Architecture (one NeuronCore per batch-pair; data-parallel over B=16 on
8 cores):

  The scan is SERIAL in t, and on real TRN2 every cross-engine hop in the
  recurrence costs ~1.4 us (semaphore+dispatch latency), so the entire
  state chain lives on the Vector engine (DVE).  The two batch items are
  interleaved as independent half-ops so every dependent same-engine pair
  is separated by an independent op (hides the SBUF write->read bubble):

  DVE : u_b  = a_b + xs_t             (tensor_tensor add, [128,512] x2)
        a_b' = (u_b - (u_b>1)) * decay (custom fused op LIF_RESET_DECAY x2)
  ACT : g = Sign(u - 1) -> {-1,+1} bf16         (spikes, off-chain)
  PE  : psum[32c:32c+32] += (W*256^kk)^T g_b    (bit-pack: 8 partitions ->
        one f32 holding 8 spike bits; 3/3/2 timesteps accumulate per slice
        at the three legal PSUM write offsets 0/32/64 -> 8 timesteps/bank)
  ACT : psum -> SBUF copy; one DMA store per group of 8 timesteps
        (16x less store traffic than storing spikes directly)

  Host decodes bits: X = (P + 255*sum(256^kk))/2 per slice, unpackbits.

Measured on HW (hardware-loop repeat-delta): ~86 us/core steady state,
~95% of which is the DVE chain floor (32 steps x 4 f32 ops + issue
overhead = 81.8 us); input DMA (16 MiB/core) fully overlaps under it.
"""

import numpy as np

B, T, C, H, W = 16, 32, 64, 32, 32
VTH_M, VTH_S, DECAY_M, DECAY_S = 0.5, 0.1, 2.0, 0.1
N_CORES = 8
B_LOC = B // N_CORES          # 2 batch items per core
P = 128
CHW = C * H * W               # 65536
FD = CHW // P                 # 512
WID = B_LOC * FD              # 1024 merged columns
GT = 8                        # timesteps packed per PSUM bank
G = T // GT                   # 4 groups
LOAD_T = 4                    # timesteps per input DMA
XP_BUFS = 6
UP_BUFS = 6
AP_BUFS = 4
GP_BUFS = 6
SP_BUFS = 4
PS_BUFS = 2

_STATE: dict = {}


# --------------------------------------------------------------------------
# Custom DVE op (registered once per process)
# --------------------------------------------------------------------------

def _get_ops():
    if "ops" in _STATE:
        return _STATE["ops"]
    from concourse import dve_ops
    from concourse.dve_spec import Spec, Src0, Src1, C0, lower, _has_src1
    from concourse.dve_uop import DveOpSpec

    def register(name, spec):
        for op in dve_ops.OPS:
            if op.name == name:
                return op
        row = dve_ops._CUSTOM_DVE_ROW_BASE + len(dve_ops.OPS)
        shas = {}
        for ver in ("v3", "v4"):
            s = DveOpSpec(
                name=name, opcode=row, uops=lower(spec, ver=ver),
                rd1_en=_has_src1(spec),
            )
            shas[ver] = s.sha(ver)
        op = dve_ops.DveOp(name, spec, subdim=False, uops_sha=shas)
        dve_ops.OPS.append(op)
        dve_ops._SUB_OPCODE_FOR_NAME[name] = row
        dve_ops.CUSTOM_DVE_SPECS[name] = spec
        return op

    # a' = (u - (u > 1)) * decay  — soft reset + leak in one DVE pass
    reset_decay = register(
        "LIF_RESET_DECAY",
        Spec(
            body=(Src0 - (Src0 > C0)) * Src1,
            reference=lambda in0, in1, s0, s1, imm2: (
                (in0.astype(np.float32) - (in0 > s0)) * in1
            ).astype(np.float32),
        ),
    )
    _STATE["ops"] = (reset_decay,)
    return _STATE["ops"]


# --------------------------------------------------------------------------
# Device kernel build
# --------------------------------------------------------------------------

def _emit_body(nc, tc, pools, tensors, reps, mybir, reset_decay, loop=False):
    f32 = mybir.dt.float32
    bf16 = mybir.dt.bfloat16
    Sign = mybir.ActivationFunctionType.Sign
    pp, xp, up, ap, gp, sp, psp = pools
    xs_d, dec_d, w_d, pk_d, dec, wpk, bias_m1 = tensors

    # First xs tiles arrive in small chunks so compute starts early.
    load_plan = [(0, 1), (1, 1), (2, 2)]
    t0n = 4
    while t0n < T:
        load_plan.append((t0n, LOAD_T))
        t0n += LOAD_T
    loads = {t0: (t0, n, i) for i, (t0, n) in enumerate(load_plan)}

    for r in range(reps):
        w = None   # zero state at t=0: u_0 == xs_0, no memset/add needed

        first = (r == 0) and not loop
        xt = {}
        for g in range(G):
            ps = [psp.tile([P, FD], f32, name=f"ps{r}_{b}_{g}",
                           tag=f"ps{b}") for b in range(B_LOC)]
            for k in range(GT):
                t = g * GT + k
                if first and t == 0:
                    # dec rides first: the t=0 reset op needs it
                    nc.sync.dma_start(dec, dec_d[:, :])
                    first = False
                if t in loads:
                    t0_, n_, li_ = loads[t]
                    xl = xp.tile([P, n_, WID], f32, name=f"x{r}_{t}", tag="x")
                    nc.sync.dma_start(xl, xs_d[:, t0_:t0_ + n_, :])
                    for j in range(n_):
                        xt[t0_ + j] = xl[:, j, :]
                    if t0_ == 0 and not loop and r == 0:
                        nc.sync.dma_start(wpk, w_d[:, :])

                # interleave the two batch-halves so each dependent
                # same-engine pair is separated by an independent op
                if t == 0:
                    ut = xt[0]
                else:
                    ut = up.tile([P, WID], f32, name=f"u{r}_{t}", tag="u")
                    for h in range(B_LOC):
                        nc.vector.tensor_tensor(
                            ut[:, h * FD:(h + 1) * FD], w[h],
                            xt[t][:, h * FD:(h + 1) * FD], mybir.AluOpType.add)
                if t < T - 1:
                    wnew = []
                    for h in range(B_LOC):
                        wn = ap.tile([P, FD], f32, name=f"wn{r}_{t}_{h}",
                                     tag=f"w{h}")
                        nc.vector._custom_dve(
                            reset_decay, out=wn,
                            in0=ut[:, h * FD:(h + 1) * FD],
                            in1=dec, s0=1.0)
                        wnew.append(wn)
                    w = wnew

                gt_ = gp.tile([P, WID], bf16, name=f"g{r}_{t}", tag="g")
                nc.scalar.activation(gt_, ut, Sign, bias=bias_m1)

                # slice c (offset 32c) accumulates timesteps kk=0..2
                # (c=2: kk=0..1) with weights W*256^kk; weight columns
                # 16..31 are zero so kk=0 initializes the full slice.
                c = k // 3 if k < 6 else 2
                kk = k % 3 if k < 6 else k - 6
                last = (kk == 2) or (k == GT - 1)
                for b in range(B_LOC):
                    nc.tensor.matmul(
                        ps[b][32 * c:32 * c + 32, :],
                        wpk[:, 32 * kk:32 * (kk + 1)],
                        gt_[:, b * FD:(b + 1) * FD],
                        start=(kk == 0), stop=last)

            for b in range(B_LOC):
                st = sp.tile([96, FD], f32, name=f"st{r}_{g}_{b}", tag="st")
                nc.scalar.copy(st, ps[b][0:96, :])
                nc.sync.dma_start(pk_d[g, :, b * FD:(b + 1) * FD], st)


def _build_nc(reps=1, loop_R=None):
    import concourse.bacc as bacc
    import concourse.mybir as mybir
    from concourse.tile import TileContext

    (reset_decay,) = _get_ops()
    f32 = mybir.dt.float32
    bf16 = mybir.dt.bfloat16

    nc = bacc.Bacc(trn_type="TRN2")
    # xs partition-major: [P, T, WID]; column block b holds batch item b.
    xs_d = nc.dram_tensor("xs", [P, T, WID], f32, kind="ExternalInput")
    dec_d = nc.dram_tensor("decay", [P, FD], f32, kind="ExternalInput")
    w_d = nc.dram_tensor("wpk", [P, 96], bf16, kind="ExternalInput")
    pk_d = nc.dram_tensor("pk", [G, 96, WID], f32, kind="ExternalOutput")

    with TileContext(nc) as tc:
        with tc.tile_pool(name="pp", bufs=1) as pp, \
             tc.tile_pool(name="xp", bufs=XP_BUFS) as xp, \
             tc.tile_pool(name="up", bufs=UP_BUFS) as up, \
             tc.tile_pool(name="ap", bufs=AP_BUFS) as ap, \
             tc.tile_pool(name="gp", bufs=GP_BUFS) as gp, \
             tc.tile_pool(name="sp", bufs=SP_BUFS) as sp, \
             tc.psum_pool(name="ps", bufs=PS_BUFS) as psp:

            dec = pp.tile([P, FD], f32, name="dec", tag="dec")
            wpk = pp.tile([P, 96], bf16, name="wpk", tag="wpk")
            bias_m1 = pp.tile([P, 1], f32, name="biasm1", tag="biasm1")
            nc.gpsimd.memset(bias_m1, -1.0)

            pools = (pp, xp, up, ap, gp, sp, psp)
            tensors = (xs_d, dec_d, w_d, pk_d, dec, wpk, bias_m1)
            if loop_R is not None:
                nc.sync.dma_start(dec, dec_d[:, :])
                nc.sync.dma_start(wpk, w_d[:, :])
                with tc.For_i(0, loop_R) as _i:
                    _emit_body(nc, tc, pools, tensors, 1, mybir, reset_decay,
                               loop=True)
            else:
                _emit_body(nc, tc, pools, tensors, reps, mybir, reset_decay)
    nc.finalize()
    return nc


def _get_nc():
    nc = _STATE.get("nc")
    if nc is None:
        nc = _build_nc()
        _STATE["nc"] = nc
    return nc


# --------------------------------------------------------------------------
# Host side
# --------------------------------------------------------------------------

def _pack_weights():
    w = np.zeros((P, 96), np.float32)
    for kk in range(3):
        for p in range(P):
            w[p, 32 * kk + p // 8] = float(2 ** (p % 8 + 8 * kk))
    return w


def _prep_inputs(x, vth_raw, decay_raw):
    import ml_dtypes
    x = np.asarray(x, dtype=np.float32)
    vth_raw = np.asarray(vth_raw, dtype=np.float32)
    decay_raw = np.asarray(decay_raw, dtype=np.float32)

    vth64 = np.logaddexp(0.0, vth_raw.astype(np.float64) * VTH_S + VTH_M) + 0.01
    dec64 = 1.0 / (1.0 + np.exp(-(decay_raw.astype(np.float64) * DECAY_S + DECAY_M)))
    dec = np.clip(dec64, 0.0, 0.99).astype(np.float32)
    ivth = (1.0 / vth64).astype(np.float32)

    xs = x * ivth[None, None]                       # (B,T,C,H,W) f32
    xs_rs = xs.reshape(B, T, P, FD)
    dec_wide = np.ascontiguousarray(dec.reshape(P, FD))
    wpk = _pack_weights().astype(ml_dtypes.bfloat16)

    in_maps = []
    for kcore in range(N_CORES):
        sh = xs_rs[kcore * B_LOC:(kcore + 1) * B_LOC]   # (B_LOC, T, P, FD)
        merged = np.ascontiguousarray(
            sh.transpose(2, 1, 0, 3).reshape(P, T, WID))
        in_maps.append({"xs": merged, "decay": dec_wide, "wpk": wpk})
    return in_maps


def _decode(pk):
    """pk (G, 96, WID) f32 packed -> spikes (B_LOC, T, P, FD) f32."""
    pk = pk.reshape(G, 3, 32, WID)[:, :, :16]         # (G, c, m, WID)
    s = np.empty((G, GT, 16, 8, WID), np.uint8)
    for c in range(3):
        n_kk = 3 if c < 2 else 2
        const = 255.0 * sum(256 ** kk for kk in range(n_kk))
        y = np.rint((pk[:, c] + const) * 0.5).astype(np.int64)
        for kk in range(n_kk):
            xb = ((y >> (8 * kk)) & 0xFF).astype(np.uint8)   # (G, 16, WID)
            bits = np.unpackbits(xb[..., None], axis=-1, bitorder="little")
            s[:, 3 * c + kk] = bits.transpose(0, 1, 3, 2)
    s = s.reshape(T, P, B_LOC, FD)                    # partition p = 8m+j
    return s.transpose(2, 0, 1, 3).astype(np.float32)


def _run(in_maps, trace=False):
    from concourse.bass_utils import run_bass_kernel_spmd
    nc = _get_nc()
    return run_bass_kernel_spmd(
        nc, in_maps, core_ids=list(range(N_CORES)), trace=trace,
    )


def _assemble(res):
    out = np.empty((B, T, C, H, W), np.float32)
    for kcore in range(N_CORES):
        pk = np.asarray(res.results[kcore]["pk"], np.float32)
        out[kcore * B_LOC:(kcore + 1) * B_LOC] = _decode(pk).reshape(
            B_LOC, T, C, H, W)
    return out


def kernel(x, vth_raw, decay_raw):
    in_maps = _prep_inputs(x, vth_raw, decay_raw)
    res = _run(in_maps, trace=False)
    return _assemble(res)


def kernel_traced(x, vth_raw, decay_raw):
    in_maps = _prep_inputs(x, vth_raw, decay_raw)
    res = _run(in_maps, trace=True)
    return _assemble(res), res


# --------------------------------------------------------------------------
# HW timing (hardware-loop repeat-delta; used by test.py, not the harness)
# --------------------------------------------------------------------------

def _make_runner(nc):
    import jax
    from jax.sharding import Mesh, PartitionSpec
    from jax.experimental.shard_map import shard_map
    import concourse.mybir as mybir
    from concourse import bass2jax

    bass2jax.install_neuronx_cc_hook()

    partition_name = nc.partition_id_tensor.name if nc.partition_id_tensor else None
    in_names, out_names, out_avals, zero_outs = [], [], [], []
    for alloc in nc.m.functions[0].allocations:
        if not isinstance(alloc, mybir.MemoryLocationSet):
            continue
        name = alloc.memorylocations[0].name
        if alloc.kind == "ExternalInput":
            if name != partition_name:
                in_names.append(name)
        elif alloc.kind == "ExternalOutput":
            shape = tuple(alloc.tensor_shape)
            dtype = mybir.dt.np(alloc.dtype)
            out_names.append(name)
            out_avals.append(jax.core.ShapedArray(shape, dtype))
            zero_outs.append(np.zeros(shape, dtype))
    n_params = len(in_names)
    n_outs = len(out_avals)
    all_in_names = list(in_names) + list(out_names)
    if partition_name is not None:
        all_in_names.append(partition_name)

    def _body(*args):
        operands = list(args)
        if partition_name is not None:
            operands.append(bass2jax.partition_id_tensor())
        outs = bass2jax._bass_exec_p.bind(
            *operands,
            out_avals=tuple(out_avals),
            in_names=tuple(all_in_names),
            out_names=tuple(out_names),
            lowering_input_output_aliases=(),
            sim_require_finite=True,
            sim_require_nnan=True,
            nc=nc,
        )
        return tuple(outs)

    devices = jax.devices()[:N_CORES]
    mesh = Mesh(np.asarray(devices), ("core",))
    in_specs = (PartitionSpec("core"),) * (n_params + n_outs)
    out_specs = (PartitionSpec("core"),) * n_outs
    sharded = jax.jit(
        shard_map(_body, mesh=mesh, in_specs=in_specs, out_specs=out_specs,
                  check_rep=False),
        keep_unused=True,
    )

    from jax.sharding import NamedSharding
    zero_sharding = NamedSharding(mesh, PartitionSpec("core"))
    zero_cache = []

    def run(concat_inputs_by_name):
        if not zero_cache:
            zero_cache.extend(
                jax.device_put(
                    np.zeros((N_CORES * z.shape[0], *z.shape[1:]), z.dtype),
                    zero_sharding,
                )
                for z in zero_outs
            )
        args = [concat_inputs_by_name[n] for n in in_names]
        args += zero_cache
        outs = sharded(*args)
        return outs, out_names

    run.mesh = mesh
    run.in_names = in_names
    run.out_names = out_names
    return run


def measure_hw_ns(x, vth_raw, decay_raw, r_lo=4, r_hi=1028, n_calls=8):
    """Steady-state per-iteration device time: the same kernel wrapped in a
    For_i hardware loop run at R=r_lo and R=r_hi; (minwall delta)/(R delta)
    cancels the ~+-15 ms axon dispatch noise (signal ~50 ms at R=516)."""
    import time
    import jax
    from jax.sharding import NamedSharding, PartitionSpec

    in_maps = _prep_inputs(x, vth_raw, decay_raw)
    concat = {
        n: np.concatenate([np.asarray(m[n]) for m in in_maps], axis=0)
        for n in in_maps[0]
    }
    mins = {}
    for R in (r_lo, r_hi):
        nc = _build_nc(loop_R=R)
        run = _make_runner(nc)
        sh = NamedSharding(run.mesh, PartitionSpec("core"))
        dev_in = {n: jax.device_put(concat[n], sh) for n in run.in_names}
        outs, _ = run(dev_in)           # warmup + compile
        jax.block_until_ready(outs)
        ts = []
        for _ in range(n_calls):
            t0 = time.perf_counter()
            outs, _ = run(dev_in)
            jax.block_until_ready(outs)
            ts.append(time.perf_counter() - t0)
        mins[R] = min(ts)
        print(f"  R={R}: min={min(ts)*1e3:.2f} ms  all={[f'{t*1e3:.1f}' for t in ts]}")
    ns = (mins[r_hi] - mins[r_lo]) / (r_hi - r_lo) * 1e9
    return ns, mins
